# revision 25
# baseline (speedup 1.0000x reference)
"""DCGRU cell on 8 Trainium2 NeuronCores.

Strategy (dst-sharded, rs-recompute pass 2, chunked fp8 AllGather):
  - Nodes are sharded into 8 contiguous ranges (one per core); within a core
    nodes are dealt into 49 blocks of 128 by in-degree (load balance). Edges
    live on the core owning dst.
  - Pass 1 gathers x1 = [feat, state] rows (bf16, 256B) from the replicated
    x1b table (lo/hi halves for int16 indices) and segment-sums them via
    one-hot matmuls into transposed PSUM agg [128 dims, 128 dst]; the agg is
    persisted in SBUF (its feat half is reused by pass 2, since
    agg2 = A[feat] @ Wc_top + A[rs] @ Wc_bot).
  - Tail A: zr = sigmoid(aggT.T @ Wzr + bzr); z^T persisted; rs = r * state
    stored to DRAM in fp8. Every ~12 blocks one AllGather chunk ships rs to
    all cores (4 chunks, each its own DRAM tensor so the chunk collectives
    and the pass-2 gathers that consume them pipeline independently).
  - Pass 2 groups edges by (dst block, src chunk). For each chunk class, as
    soon as its collective lands, single-row rs values are gathered with
    elem_size=256B/elem_step=64B (row + 3 ignored neighbor rows), upconverted
    fp8->bf16, and accumulated via one-hot matmuls into [64, 128] PSUM, then
    added into an SBUF accumulator accT. After a block's last class:
    c^T = tanh(Wc_top^T @ A[f]^T + Wc_bot^T @ accT + bc); new_state^T =
    z^T * (state^T - c^T) + c^T, stored transposed (512B lines, no penalty).
"""

import numpy as np

import concourse.bass as bass
import concourse.bacc as bacc
import concourse.mybir as mybir
import concourse.tile as tile
from concourse.bass_utils import run_bass_kernel_spmd
from concourse.library_config import mlp
from concourse.masks import make_identity

N_NODES = 50000
N_EDGES = 640000
HID = 64
N_CORES = 8
BLK = 128            # dst nodes per block (= PSUM partition dim)
SB_BLOCKS = 4        # dst blocks per super-block (pass-1 gather scope)
MAX_G1 = 26          # cap on groups per dma_gather instruction (pass 1)
MAX_G2 = 26          # cap on groups per dma_gather instruction (pass 2)
CH_SPLIT = [0, 17, 33, 49]   # chunk boundaries in blocks (3 chunks)
N_CH = 3

F32 = mybir.dt.float32
BF16 = mybir.dt.bfloat16
FP8 = mybir.dt.float8e4
I16 = mybir.dt.int16

RS_DT = FP8          # collective payload dtype (upconverted to bf16 pair tables)


def _ceil16(x):
    return max(16, ((int(x) + 15) // 16) * 16)


def _prep_edges(dst, src, edge_weight, n_nodes, n_cores):
    """Partition edges by dst core/block; build pass-1 (src half) and pass-2
    (src chunk) group tables. Returns (tables, plan)."""
    shard = n_nodes // n_cores
    nblk = (shard + BLK - 1) // BLK
    split = n_nodes // 2
    e = len(dst)

    ch_rows = [(CH_SPLIT[c + 1] - CH_SPLIT[c]) * BLK for c in range(N_CH)]
    ch_rows[-1] = shard - CH_SPLIT[N_CH - 1] * BLK  # last chunk partial block
    ch_of_block = np.zeros(nblk, np.int64)
    for c in range(N_CH):
        ch_of_block[CH_SPLIT[c]:CH_SPLIT[c + 1]] = c

    dsts = dst.astype(np.int64)
    srcs = src.astype(np.int64)
    owner = dsts // shard
    local = dsts - owner * shard

    # Balance in-degree across blocks: per core, deal nodes (sorted by
    # in-degree, desc) round-robin over blocks.
    deg = np.zeros(n_nodes, np.int64)
    np.add.at(deg, dsts, 1)
    pos = np.empty((n_cores, shard), np.int64)
    blk_fill = np.empty(nblk, np.int64)
    cap = np.full(nblk, BLK, np.int64)
    cap[nblk - 1] = shard - (nblk - 1) * BLK
    for p in range(n_cores):
        nodes = np.argsort(-deg[p * shard : (p + 1) * shard], kind="stable")
        blk_fill[:] = 0
        bi = 0
        for n in nodes:
            while blk_fill[bi % nblk] >= cap[bi % nblk]:
                bi += 1
            b = bi % nblk
            pos[p, n] = b * BLK + blk_fill[b]
            blk_fill[b] += 1
            bi += 1
    newloc = pos[owner, local]
    b_of = newloc // BLK
    dloc = (newloc % BLK).astype(np.float64)

    src_owner = srcs // shard
    src_l = pos[src_owner, srcs % shard]          # permuted local row of src
    src_pos = src_owner * shard + src_l           # global row in x1b
    cls1 = (src_pos >= split).astype(np.int64)
    src_local1 = src_pos - cls1 * split           # pass-1 idx (< 25000)

    src_ch = ch_of_block[src_l // BLK]            # pass-2 class (src chunk)
    row2 = src_owner * np.array(ch_rows)[src_ch] + (
        src_l - np.array(CH_SPLIT)[src_ch] * BLK
    )                                             # pass-2 idx within chunk tbl

    w64 = edge_weight.astype(np.float64)

    def build_pass(cls, idxval, n_cls, unit_order, force_units, par=None):
        """Group edges into (block, cls) units following unit_order.

        Returns idx16 [cores,16,8*ng], dst_t, w_t [cores,128,ng] (f64),
        chunks [(g0,g1,cls,nidx)], blk_of_g, first_g/last_g dicts keyed
        (b, cls), ngroups.
        """
        cnt = np.zeros((n_cores, nblk, n_cls), np.int64)
        np.add.at(cnt, (owner, b_of, cls), 1)
        unit_max = cnt.max(axis=0)                   # [nblk, n_cls]
        gp = -(-unit_max // BLK)                     # groups per unit
        for (b, c) in force_units:
            gp[b, c] = max(gp[b, c], 1)

        unit_sizes = np.array([gp[b, c] for (b, c) in unit_order], np.int64)
        unit_off = np.concatenate([[0], np.cumsum(unit_sizes)])
        ngroups = int(unit_off[-1])
        unit_idx = {bc: i for i, bc in enumerate(unit_order)}

        blk_of_g = np.zeros(ngroups, np.int64)
        cls_of_u = {}
        for i, (b, c) in enumerate(unit_order):
            blk_of_g[unit_off[i] : unit_off[i + 1]] = b
            cls_of_u[(b, c)] = i
        first_g = {}
        last_g = {}
        for i, (b, c) in enumerate(unit_order):
            if gp[b, c] > 0:
                first_g[(b, c)] = int(unit_off[i])
                last_g[(b, c)] = int(unit_off[i + 1]) - 1

        # number of real idxs in each unit (rounded up to 16)
        unit_n16 = np.array(
            [_ceil16(unit_max[b, c]) if gp[b, c] > 0 else 0 for (b, c) in unit_order],
            np.int64,
        )
        # clamp to group capacity
        unit_n16 = np.minimum(unit_n16, unit_sizes * BLK)

        # gather chunks: runs of same class in unit_order, capped; slots past
        # the last real edge of a chunk's tail unit are trimmed off num_idxs
        # (un-gathered slots stay stale in SBUF; their one-hot weights are 0).
        maxg = MAX_G1 if n_cls == 2 else MAX_G2
        chunks = []
        i = 0
        while i < len(unit_order):
            c = unit_order[i][1]
            j = i
            while j < len(unit_order) and unit_order[j][1] == c:
                j += 1
            g_run1 = int(unit_off[j]) if j < len(unit_order) else ngroups
            s = int(unit_off[i])
            while s < g_run1:
                t = min(s + maxg, g_run1)
                u = int(np.searchsorted(unit_off, t - 1, side="right") - 1)
                if t == unit_off[u + 1]:
                    # chunk ends at unit u's end: drop u's tail padding
                    lu0 = max(int(unit_off[u]), s)
                    done_before = (lu0 - int(unit_off[u])) * BLK
                    tail = int(unit_n16[u]) - done_before
                    tail = max(16, min(tail, (t - lu0) * BLK))
                    nidx = (lu0 - s) * BLK + tail
                else:
                    nidx = (t - s) * BLK
                chunks.append((int(s), int(t), int(c), int(_ceil16(nidx))))
                s = t
            i = j

        # slot assignment: rank within (core, unit)
        ukey = np.array([unit_idx[(b, c)] for b, c in zip(b_of, cls)], np.int64)
        ck = owner * len(unit_order) + ukey
        order2 = np.argsort(ck, kind="stable")
        ck_s = ck[order2]
        owner_s = owner[order2]
        idx_s = idxval[order2]
        ukey_s = ukey[order2]
        dloc_s = dloc[order2]
        ws_s = w64[order2]
        bucket_start = np.searchsorted(ck_s, np.arange(n_cores * len(unit_order)))
        rank = np.arange(e) - bucket_start[ck_s]
        g_global = unit_off[ukey_s] + rank // BLK
        lane = rank % BLK

        idx16 = np.zeros((n_cores, 16, 8 * ngroups), np.int16)
        dst_t = np.zeros((n_cores, BLK, ngroups), np.float64)
        w_t = np.zeros((n_cores, BLK, ngroups), np.float64)
        idx16[owner_s, lane % 16, 8 * g_global + lane // 16] = idx_s.astype(np.int16)
        if par is None:
            dst_t[owner_s, lane, g_global] = dloc_s
        else:
            dst_t[owner_s, lane, g_global] = dloc_s + BLK * par[order2]
        w_t[owner_s, lane, g_global] = ws_s
        return {
            "idx16": idx16,
            "dst_t": dst_t,
            "w_t": w_t,
            "chunks": chunks,
            "blk_of_g": [int(x) for x in blk_of_g],
            "first_g": first_g,
            "last_g": last_g,
            "ngroups": ngroups,
            "gp": gp,
        }

    # ---- pass 1: units (sb, half, block) ----
    unit_order1 = []
    for sb0 in range(0, nblk, SB_BLOCKS):
        sbb = range(sb0, min(sb0 + SB_BLOCKS, nblk))
        for c in range(2):
            for b in sbb:
                unit_order1.append((b, c))
    force1 = [(b, 0) for b in range(nblk)]
    p1 = build_pass(cls1, src_local1, 2, unit_order1, force1)

    # ---- pass 2 (packed): slots packed at 16-granularity; groups span
    # blocks, with one one-hot emission per (group, block) segment ----
    def build_pass2_packed(cls, idxval, par):
        cnt = np.zeros((n_cores, nblk, N_CH), np.int64)
        np.add.at(cnt, (owner, b_of, cls), 1)
        unit_max = cnt.max(axis=0)                  # [nblk, N_CH]
        n16 = np.zeros((nblk, N_CH), np.int64)
        for b in range(nblk):
            for c in range(N_CH):
                if unit_max[b, c] > 0 or c == 0:
                    n16[b, c] = _ceil16(unit_max[b, c])

        unit_order = []
        for c in range(N_CH):
            for sb0 in range(0, nblk, SB_BLOCKS):
                for b in range(sb0, min(sb0 + SB_BLOCKS, nblk)):
                    if n16[b, c] > 0:
                        unit_order.append((b, c))
        # pack units; round each class's span up to 128 slots
        unit_base = {}
        slot = 0
        cls_span = {}
        cur_c = None
        for (b, c) in unit_order + [(None, None)]:
            if c != cur_c:
                if cur_c is not None:
                    slot = -(-slot // BLK) * BLK
                    cls_span[cur_c] = (cls_span[cur_c][0], slot)
                if c is not None:
                    cls_span[c] = (slot, None)
                cur_c = c
            if b is not None:
                unit_base[(b, c)] = slot
                slot += n16[b, c]
        nslots = slot
        ngroups = nslots // BLK

        # emissions: per group, per (block-segment)
        emit_of = {}
        emissions = [[] for _ in range(ngroups)]
        n_emit = 0
        for (b, c) in unit_order:
            s0, s1 = unit_base[(b, c)], unit_base[(b, c)] + n16[b, c]
            for g in range(s0 // BLK, (s1 - 1) // BLK + 1):
                emit_of[(g, b)] = n_emit
                emissions[g].append([b, c, n_emit, False, False])
                n_emit += 1
        # start/stop flags per (b, c)
        for (b, c) in unit_order:
            s0, s1 = unit_base[(b, c)], unit_base[(b, c)] + n16[b, c]
            g0u, g1u = s0 // BLK, (s1 - 1) // BLK
            for em in emissions[g0u]:
                if em[0] == b and em[1] == c:
                    em[3] = True
            for em in emissions[g1u]:
                if em[0] == b and em[1] == c:
                    em[4] = True

        # gather chunks per class, trimmed at class tails
        chunks = []
        for c in range(N_CH):
            s0, s1 = cls_span[c]
            real_end = max(
                unit_base[(b, cc)] + n16[b, cc]
                for (b, cc) in unit_order
                if cc == c
            )
            g0c, g1c = s0 // BLK, s1 // BLK
            s = g0c
            while s < g1c:
                t = min(s + MAX_G2, g1c)
                nidx = min(t * BLK, real_end) - s * BLK
                chunks.append((s, t, c, int(_ceil16(max(16, nidx)))))
                s = t

        # slot assignment: rank within (core, unit)
        uidx = {bc: i for i, bc in enumerate(unit_order)}
        ukey = np.array([uidx[(b, c)] for b, c in zip(b_of, cls)], np.int64)
        ck = owner * len(unit_order) + ukey
        order2 = np.argsort(ck, kind="stable")
        ck_s = ck[order2]
        owner_s = owner[order2]
        idx_s = idxval[order2]
        dloc_s = dloc[order2]
        ws_s = w64[order2]
        par_s = par[order2]
        base_arr = np.array([unit_base[bc] for bc in unit_order], np.int64)
        bucket_start = np.searchsorted(ck_s, np.arange(n_cores * len(unit_order)))
        rank = np.arange(e) - bucket_start[ck_s]
        slot_arr = base_arr[ukey_s := ukey[order2]] + rank
        g_arr = slot_arr // BLK
        lane = slot_arr % BLK
        col = np.array(
            [emit_of[(int(g), int(b))] for g, b in zip(g_arr, b_of[order2])],
            np.int64,
        )

        idx16 = np.zeros((n_cores, 16, 8 * ngroups), np.int16)
        dst_t = np.zeros((n_cores, BLK, n_emit), np.float64)
        w_t = np.zeros((n_cores, BLK, n_emit), np.float64)
        idx16[owner_s, lane % 16, 8 * g_arr + lane // 16] = idx_s.astype(np.int16)
        dst_t[owner_s, lane, col] = dloc_s + BLK * par_s
        w_t[owner_s, lane, col] = ws_s
        return {
            "idx16": idx16,
            "dst_t": dst_t,
            "w_t": w_t,
            "chunks": chunks,
            "emissions": emissions,
            "ngroups": ngroups,
            "n_emit": n_emit,
            "n16": n16,
        }

    p2 = build_pass2_packed(src_ch, row2 // 2, (row2 % 2))

    last_cls = np.zeros(nblk, np.int64)
    for b in range(nblk):
        for c in range(N_CH):
            if p2["n16"][b, c] > 0:
                last_cls[b] = c

    plan = {
        "p1": p1,
        "p2": p2,
        "nblk": nblk,
        "shard": shard,
        "ch_rows": ch_rows,
        "last_cls": [int(x) for x in last_cls],
        "pos": pos,
    }
    return plan


def _build(n_nodes, hid, plan, n_cores, n_queues=4):
    """Build the SPMD Bass program from the edge plan."""
    shard = plan["shard"]
    nblk = plan["nblk"]
    p1, p2 = plan["p1"], plan["p2"]
    ch_rows = plan["ch_rows"]
    last_cls = plan["last_cls"]
    split = n_nodes // 2
    h2 = 2 * hid
    ng1, ng2 = p1["ngroups"], p2["ngroups"]
    n_emit = p2["n_emit"]
    npad = nblk * BLK  # 6272

    nc = bacc.Bacc(None, num_devices=n_cores, num_swdge_queues=n_queues)

    x1b = nc.dram_tensor("x1b", [n_nodes, h2], BF16, kind="ExternalInput")
    st_d = nc.dram_tensor("st_d", [npad, hid], BF16, kind="ExternalInput")
    stT_d = nc.dram_tensor("stT_d", [hid, npad], BF16, kind="ExternalInput")
    idx1_d = nc.dram_tensor("idx1", [BLK, 8 * ng1], I16, kind="ExternalInput")
    idx2_d = nc.dram_tensor("idx2", [BLK, 8 * ng2], I16, kind="ExternalInput")
    dst1_d = nc.dram_tensor("dst1", [BLK, ng1], F32, kind="ExternalInput")
    w1_d = nc.dram_tensor("w1", [BLK, ng1], F32, kind="ExternalInput")
    dst2_d = nc.dram_tensor("dst2", [BLK, n_emit], F32, kind="ExternalInput")
    w2_d = nc.dram_tensor("w2", [BLK, n_emit], F32, kind="ExternalInput")
    wzr = nc.dram_tensor("wzr", [h2, h2], F32, kind="ExternalInput")
    bzr = nc.dram_tensor("bzr", [1, h2], F32, kind="ExternalInput")
    wc = nc.dram_tensor("wc", [h2, hid], F32, kind="ExternalInput")
    bc = nc.dram_tensor("bc", [1, hid], F32, kind="ExternalInput")
    outT = nc.dram_tensor("outT", [hid, npad], F32, kind="ExternalOutput")

    rs_sh = [
        nc.dram_tensor(f"rs_sh{c}", [ch_rows[c], hid], RS_DT, kind="Internal")
        for c in range(N_CH)
    ]
    ch_real = [n_cores * ch_rows[c] for c in range(N_CH)]
    ch_alloc = [-(-r // BLK) * BLK for r in ch_real]
    rs_full = [
        nc.dram_tensor(
            f"rs_full{c}", [ch_alloc[c], hid], RS_DT,
            kind="Internal", addr_space="Shared",
        )
        for c in range(N_CH)
    ]
    rs_pair = [
        nc.dram_tensor(
            f"rs_pair{c}", [ch_alloc[c] + 2, hid], BF16, kind="Internal"
        )
        for c in range(N_CH)
    ]

    qn = [0]

    def next_q():
        q = qn[0]
        qn[0] = (qn[0] + 1) % n_queues
        return q

    def rows_of(b):
        return BLK if b < nblk - 1 else shard - (nblk - 1) * BLK

    with tile.TileContext(nc) as tc:
        with (
            tc.tile_pool(name="const", bufs=1) as const_pool,
            tc.tile_pool(name="store", bufs=1) as store_pool,
            tc.tile_pool(name="msg", bufs=3) as msg_pool,
            tc.tile_pool(name="oh", bufs=6) as oh_pool,
            tc.tile_pool(name="blk", bufs=6) as blk_pool,
            tc.tile_pool(name="stage", bufs=2) as stage_pool,
            tc.tile_pool(name="agg_ps", bufs=4, space="PSUM") as agg_psum,
            tc.tile_pool(name="agg2_ps", bufs=2, space="PSUM") as agg2_psum,
            tc.tile_pool(name="mm_ps", bufs=2, space="PSUM") as mm_psum,
        ):
            nc.gpsimd.load_library(mlp)
            # ---- constants ----
            iota_i = const_pool.tile([BLK, BLK], mybir.dt.int32)
            nc.gpsimd.iota(iota_i[:], pattern=[[1, BLK]], base=0, channel_multiplier=0)
            iota_h = const_pool.tile([BLK, BLK], BF16)
            nc.vector.tensor_copy(iota_h[:], iota_i[:])
            iota2_i = const_pool.tile([BLK, 2 * BLK], mybir.dt.int32)
            nc.gpsimd.iota(
                iota2_i[:], pattern=[[1, 2 * BLK]], base=0, channel_multiplier=0
            )
            iota2_h = const_pool.tile([BLK, 2 * BLK], BF16)
            nc.vector.tensor_copy(iota2_h[:], iota2_i[:])
            identity = const_pool.tile([BLK, BLK], F32)
            make_identity(nc, identity[:])
            ones1 = const_pool.tile([1, BLK], F32)
            nc.vector.memset(ones1[:], 1.0)
            wzr_sb = const_pool.tile([h2, h2], F32)
            nc.sync.dma_start(out=wzr_sb[:], in_=wzr[:, :])
            bzr_sb = const_pool.tile([1, h2], F32)
            nc.sync.dma_start(out=bzr_sb[:], in_=bzr[:, :])
            wctop_sb = const_pool.tile([hid, hid], F32)
            nc.sync.dma_start(out=wctop_sb[:], in_=wc[0:hid, :])
            wcbot_sb = const_pool.tile([hid, hid], F32)
            nc.sync.dma_start(out=wcbot_sb[:], in_=wc[hid:h2, :])
            bc_sb = const_pool.tile([1, hid], F32)
            nc.sync.dma_start(out=bc_sb[:], in_=bc[:, :])

            # ---- persistent tables (pass-1 quarters first so gathers can
            # start ~2us in; the rest stream behind them on the DMA queue) ----
            idx1_sb = store_pool.tile([BLK, 8 * ng1], I16)
            dst1_sb = store_pool.tile([BLK, ng1], F32)
            w1_sb = store_pool.tile([BLK, ng1], F32)

            def rep128(tile_sb, c0, c1):
                pass

            q1 = [0] + [min(ng1, (ng1 * (q + 1)) // 4) for q in range(4)]
            nc.sync.dma_start(
                out=idx1_sb[:, : 8 * q1[1]], in_=idx1_d[:, : 8 * q1[1]]
            )
            nc.sync.dma_start(out=dst1_sb[:, : q1[1]], in_=dst1_d[:, : q1[1]])
            nc.sync.dma_start(out=w1_sb[:, : q1[1]], in_=w1_d[:, : q1[1]])

            st_store = store_pool.tile([BLK, nblk * hid], BF16)
            nc.vector.memset(st_store[:], 0.0)
            stT_store = store_pool.tile([hid, npad], BF16)

            def late_loads2(ci):
                if ci == 2:
                    nc.sync.dma_start(
                        out=st_store[:].rearrange("l (b h) -> l b h", h=hid),
                        in_=st_d[:, :].rearrange("(b l) h -> l b h", l=BLK),
                    )
                elif ci == 12:
                    nc.sync.dma_start(out=stT_store[:], in_=stT_d[:, :])
            idx2_sb = store_pool.tile([BLK, 8 * ng2], I16)
            dst2_sb = store_pool.tile([BLK, n_emit], F32)
            w2_sb = store_pool.tile([BLK, n_emit], F32)

            def late_loads(ci):
                if ci in (1, 3, 5):
                    q = (ci + 1) // 2
                    a, b = q1[q], q1[q + 1]
                    nc.sync.dma_start(
                        out=idx1_sb[:, 8 * a : 8 * b], in_=idx1_d[:, 8 * a : 8 * b]
                    )
                    nc.sync.dma_start(out=dst1_sb[:, a:b], in_=dst1_d[:, a:b])
                    nc.sync.dma_start(out=w1_sb[:, a:b], in_=w1_d[:, a:b])
                elif ci == 14:
                    nc.sync.dma_start(out=idx2_sb[:], in_=idx2_d[:, :])
                elif ci == 16:
                    nc.sync.dma_start(out=dst2_sb[:], in_=dst2_d[:, :])
                    nc.sync.dma_start(out=w2_sb[:], in_=w2_d[:, :])

            zpad = const_pool.tile([BLK, hid], RS_DT)
            nc.vector.memset(zpad[:], 0.0)
            for c in range(N_CH):
                if ch_alloc[c] > ch_real[c]:
                    nc.sync.dma_start(
                        out=rs_full[c][ch_real[c] : ch_alloc[c], :],
                        in_=zpad[: ch_alloc[c] - ch_real[c], :],
                    )

            aggT_store = store_pool.tile([h2, npad], F32)
            accT = store_pool.tile([hid, npad], F32)
            zT_store = store_pool.tile([hid, npad], F32)

            psum_of = {}

            def do_collective(ch):
                nc.gpsimd.collective_compute(
                    "AllGather",
                    mybir.AluOpType.bypass,
                    replica_groups=[list(range(n_cores))],
                    ins=[rs_sh[ch][:, :]],
                    outs=[rs_full[ch][0 : ch_real[ch], :]],
                )

            # ============== Phase A: pass-1 aggregation ===============
            def drain_a(b):
                nc.scalar.activation(
                    aggT_store[:, b * BLK : (b + 1) * BLK],
                    psum_of.pop(b)[:],
                    mybir.ActivationFunctionType.Copy,
                )

            def tail_a(b):
                R = rows_of(b)
                zr_ps = mm_psum.tile([BLK, h2], F32, tag="mm")
                nc.tensor.matmul(
                    zr_ps[:],
                    lhsT=aggT_store[:, b * BLK : (b + 1) * BLK],
                    rhs=wzr_sb[:],
                    start=True,
                    stop=False,
                )
                nc.tensor.matmul(
                    zr_ps[:], lhsT=ones1[:], rhs=bzr_sb[:], start=False, stop=True
                )
                zr_sb = blk_pool.tile([BLK, h2], F32, tag="zr")
                nc.scalar.activation(
                    zr_sb[:], zr_ps[:], mybir.ActivationFunctionType.Sigmoid
                )
                ztp = mm_psum.tile([hid, BLK], F32, tag="mm")
                nc.tensor.transpose(
                    out=ztp[:], in_=zr_sb[:, 0:hid], identity=identity[:]
                )
                nc.scalar.activation(
                    zT_store[:, b * BLK : (b + 1) * BLK],
                    ztp[:],
                    mybir.ActivationFunctionType.Copy,
                )
                rs = blk_pool.tile([BLK, hid], RS_DT, tag="rs")
                nc.vector.tensor_tensor(
                    out=rs[:],
                    in0=zr_sb[:, hid:h2],
                    in1=st_store[:, b * hid : (b + 1) * hid],
                    op=mybir.AluOpType.mult,
                )
                ch = 0
                while b >= CH_SPLIT[ch + 1]:
                    ch += 1
                r0 = (b - CH_SPLIT[ch]) * BLK
                nc.scalar.dma_start(out=rs_sh[ch][r0 : r0 + R, :], in_=rs[:R, :])
                if b == CH_SPLIT[ch + 1] - 1:
                    do_collective(ch)

            blk1 = p1["blk_of_g"]
            f1 = {}
            l1 = {}
            for b in range(nblk):
                gs = [
                    p1["first_g"].get((b, c)) for c in range(2) if (b, c) in p1["first_g"]
                ]
                ge = [
                    p1["last_g"].get((b, c)) for c in range(2) if (b, c) in p1["last_g"]
                ]
                f1[b] = min(gs)
                l1[b] = max(ge)

            pend_a = []
            for ci, (g0, g1, c, nidx) in enumerate(p1["chunks"]):
                kg = g1 - g0
                if ci < 3:
                    nidx = kg * BLK
                late_loads(ci)
                late_loads2(ci)
                tbl = x1b[0:split, :] if c == 0 else x1b[split:n_nodes, :]
                msgs = msg_pool.tile([BLK, MAX_G1 * h2], BF16, tag="m1")
                out_ap = msgs[:, : kg * h2].rearrange("p (t w) -> p t w", w=h2)
                nc.gpsimd.dma_gather(
                    out_ap,
                    tbl,
                    idx1_sb[:, 8 * g0 : 8 * g0 + nidx // 16],
                    nidx,
                    nidx,
                    h2,
                    queue_num=next_q(),
                    single_packet=False,
                )
                for g in range(g0, g1):
                    b = blk1[g]
                    if b not in psum_of:
                        psum_of[b] = agg_psum.tile(
                            [h2, BLK], F32, tag="agg", name=f"agga{b}"
                        )
                    oh = oh_pool.tile([BLK, BLK], BF16, tag="oh")
                    nc.vector.tensor_scalar(
                        out=oh[:],
                        in0=iota_h[:],
                        scalar1=dst1_sb[:, g : g + 1],
                        scalar2=w1_sb[:, g : g + 1],
                        op0=mybir.AluOpType.is_equal,
                        op1=mybir.AluOpType.mult,
                    )
                    gl = (g - g0) * h2
                    nc.tensor.matmul(
                        out=psum_of[b][:],
                        lhsT=msgs[:, gl : gl + h2],
                        rhs=oh[:],
                        start=(g == f1[b]),
                        stop=(g == l1[b]),
                    )
                    if g == l1[b]:
                        drain_a(b)
                        pend_a.append(b)
                        if len(pend_a) > 2 * SB_BLOCKS:
                            tail_a(pend_a.pop(0))
            while pend_a:
                tail_a(pend_a.pop(0))

            # ---- upconvert fp8 rs_full -> bf16 pair tables (Act converts;
            # SP stalls on collective sems are harmless here) ----
            for c in range(N_CH):
                qs = [0]
                for q in range(8):
                    qs.append((ch_alloc[c] * (q + 1) // 8 // BLK) * BLK)
                qs[-1] = ch_alloc[c]
                for q in range(8):
                    r0, r1 = qs[q], qs[q + 1]
                    w = (r1 - r0) * hid // BLK
                    sf8 = stage_pool.tile([BLK, 2176], RS_DT, tag="sf8")
                    nc.sync.dma_start(
                        out=sf8[:, :w],
                        in_=rs_full[c][r0:r1, :].rearrange(
                            "(p r) h -> p (r h)", p=BLK
                        ),
                    )
                    sbf = stage_pool.tile([BLK, 2176], BF16, tag="sbf")
                    nc.scalar.activation(
                        sbf[:, :w], sf8[:, :w], mybir.ActivationFunctionType.Copy
                    )
                    nc.sync.dma_start(
                        out=rs_pair[c][r0:r1, :].rearrange(
                            "(p r) h -> p (r h)", p=BLK
                        ),
                        in_=sbf[:, :w],
                    )

            # ============== Phase C: pass-2 aggregation + output ===========
            def tail_c(b):
                R = rows_of(b)
                cps = mm_psum.tile([hid, BLK], F32, tag="mm")
                nc.tensor.matmul(
                    cps[:],
                    lhsT=wctop_sb[:],
                    rhs=aggT_store[0:hid, b * BLK : (b + 1) * BLK],
                    start=True,
                    stop=False,
                )
                nc.tensor.matmul(
                    cps[:],
                    lhsT=wcbot_sb[:],
                    rhs=accT[:, b * BLK : (b + 1) * BLK],
                    start=False,
                    stop=False,
                )
                nc.tensor.matmul(
                    cps[:], lhsT=bc_sb[:], rhs=ones1[:], start=False, stop=True
                )
                cT = blk_pool.tile([hid, BLK], F32, tag="cT")
                nc.scalar.activation(
                    cT[:], cps[:], mybir.ActivationFunctionType.Tanh
                )
                t1 = blk_pool.tile([hid, BLK], F32, tag="t1")
                nc.vector.tensor_tensor(
                    out=t1[:],
                    in0=stT_store[:, b * BLK : (b + 1) * BLK],
                    in1=cT[:],
                    op=mybir.AluOpType.subtract,
                )
                t2 = blk_pool.tile([hid, BLK], F32, tag="t2")
                nc.gpsimd.tensor_tensor(
                    out=t2[:],
                    in0=t1[:],
                    in1=zT_store[:, b * BLK : (b + 1) * BLK],
                    op=mybir.AluOpType.mult,
                )
                nsT = blk_pool.tile([hid, BLK], F32, tag="nsT")
                nc.vector.tensor_tensor(
                    out=nsT[:], in0=t2[:], in1=cT[:], op=mybir.AluOpType.add
                )
                nc.scalar.dma_start(
                    out=outT[:, b * BLK : b * BLK + R], in_=nsT[:, :R]
                )

            emissions = p2["emissions"]
            psum2 = {}
            pend_c = []
            acc_init = set()

            for ci, (g0, g1, c2, nidx) in enumerate(p2["chunks"]):
                kg = g1 - g0
                if ci < 3:
                    nidx = kg * BLK
                msgs2 = msg_pool.tile([BLK, MAX_G2 * h2], BF16, tag="m2")
                out_ap = msgs2[:, : kg * h2].rearrange("p (t w) -> p t w", w=h2)
                nc.gpsimd.dma_gather(
                    out_ap,
                    rs_pair[c2][:, :].rearrange("(a b) h -> a (b h)", b=2),
                    idx2_sb[:, 8 * g0 : 8 * g0 + nidx // 16],
                    nidx,
                    nidx,
                    h2,
                    queue_num=next_q(),
                    single_packet=False,
                )
                for g in range(g0, g1):
                    gl = (g - g0) * h2
                    for b, ec, col, e_start, e_stop in emissions[g]:
                        key = (b, ec)
                        if key not in psum2:
                            psum2[key] = agg2_psum.tile(
                                [hid, BLK], F32, tag="agg2", name=f"aggc{b}_{ec}"
                            )
                        oh = oh_pool.tile([BLK, 2 * BLK], BF16, tag="oh2")
                        eng = nc.gpsimd if col % 4 == 3 else nc.vector
                        eng.tensor_scalar(
                            out=oh[:],
                            in0=iota2_h[:],
                            scalar1=dst2_sb[:, col : col + 1],
                            scalar2=w2_sb[:, col : col + 1],
                            op0=mybir.AluOpType.is_equal,
                            op1=mybir.AluOpType.mult,
                        )
                        nc.tensor.matmul(
                            out=psum2[key][:],
                            lhsT=msgs2[:, gl : gl + hid],
                            rhs=oh[:, 0:BLK],
                            start=e_start,
                            stop=False,
                        )
                        nc.tensor.matmul(
                            out=psum2[key][:],
                            lhsT=msgs2[:, gl + hid : gl + h2],
                            rhs=oh[:, BLK : 2 * BLK],
                            start=False,
                            stop=e_stop,
                        )
                        if e_stop:
                            ps = psum2.pop(key)
                            if b not in acc_init:
                                acc_init.add(b)
                                nc.scalar.activation(
                                    accT[:, b * BLK : (b + 1) * BLK],
                                    ps[:],
                                    mybir.ActivationFunctionType.Copy,
                                )
                            else:
                                nc.vector.tensor_tensor(
                                    out=accT[:, b * BLK : (b + 1) * BLK],
                                    in0=ps[:],
                                    in1=accT[:, b * BLK : (b + 1) * BLK],
                                    op=mybir.AluOpType.add,
                                )
                            if ec == last_cls[b]:
                                pend_c.append(b)
                                if len(pend_c) > 8:
                                    tail_c(pend_c.pop(0))
            while pend_c:
                tail_c(pend_c.pop(0))

    nc.finalize()
    return nc


def run(feat, state, src, dst, edge_weight, Wzr, bzr, Wc, bc, trace=False):
    """Build + run on 8 cores; returns (new_state, BassKernelResults)."""
    import ml_dtypes

    n_nodes, hid = feat.shape
    n_cores = N_CORES
    shard = n_nodes // n_cores

    plan = _prep_edges(dst, src, edge_weight, n_nodes, n_cores)
    pos = plan["pos"]
    nblk = plan["nblk"]
    npad = nblk * BLK
    p1, p2 = plan["p1"], plan["p2"]

    # global permutation: node (p, l) lives at row p*shard + pos[p, l]
    inv = np.empty((n_cores, shard), np.int64)
    for p in range(n_cores):
        inv[p, pos[p]] = np.arange(shard)
    x1 = np.concatenate([feat, state], axis=1)
    x1p = np.empty_like(x1)
    for p in range(n_cores):
        x1p[p * shard : (p + 1) * shard] = x1[p * shard : (p + 1) * shard][inv[p]]
    x1b = np.ascontiguousarray(x1p.astype(ml_dtypes.bfloat16))

    nc = _build(n_nodes, hid, plan, n_cores)

    in_maps = []
    for p in range(n_cores):
        st_p = state[p * shard : (p + 1) * shard][inv[p]].astype(ml_dtypes.bfloat16)
        st_pad = np.zeros((npad, hid), ml_dtypes.bfloat16)
        st_pad[:shard] = st_p
        stT_pad = np.zeros((hid, npad), ml_dtypes.bfloat16)
        stT_pad[:, :shard] = st_p.T
        in_maps.append(
            {
                "x1b": x1b,
                "st_d": np.ascontiguousarray(st_pad),
                "stT_d": np.ascontiguousarray(stT_pad),
                "idx1": np.ascontiguousarray(np.tile(p1["idx16"][p], (8, 1))),
                "idx2": np.ascontiguousarray(np.tile(p2["idx16"][p], (8, 1))),
                "dst1": np.ascontiguousarray(p1["dst_t"][p].astype(np.float32)),
                "w1": np.ascontiguousarray(p1["w_t"][p].astype(np.float32)),
                "dst2": np.ascontiguousarray(p2["dst_t"][p].astype(np.float32)),
                "w2": np.ascontiguousarray(p2["w_t"][p].astype(np.float32)),
                "wzr": np.ascontiguousarray(Wzr, dtype=np.float32),
                "bzr": np.ascontiguousarray(bzr.reshape(1, -1), dtype=np.float32),
                "wc": np.ascontiguousarray(Wc, dtype=np.float32),
                "bc": np.ascontiguousarray(bc.reshape(1, -1), dtype=np.float32),
            }
        )

    res = run_bass_kernel_spmd(
        nc, in_maps, core_ids=list(range(n_cores)), trace=trace
    )
    shards = [
        res.results[p]["outT"][:, :shard].T[pos[p]] for p in range(n_cores)
    ]
    return np.concatenate(shards, axis=0), res


def kernel(feat, state, src, dst, edge_weight, Wzr, bzr, Wc, bc):
    out, _ = run(feat, state, src, dst, edge_weight, Wzr, bzr, Wc, bc, trace=False)
    return out


# revision 26
# speedup vs baseline: 1.0774x; 1.0774x over previous
"""DCGRU cell on 8 Trainium2 NeuronCores.

Strategy (dst-sharded, rs-recompute pass 2, chunked fp8 AllGather):
  - Nodes are sharded into 8 contiguous ranges (one per core); within a core
    nodes are dealt into 49 blocks of 128 by in-degree (load balance). Edges
    live on the core owning dst.
  - Pass 1 gathers x1 = [feat, state] rows (bf16, 256B) from the replicated
    x1b table (lo/hi halves for int16 indices) and segment-sums them via
    one-hot matmuls into transposed PSUM agg [128 dims, 128 dst]; the agg is
    persisted in SBUF (its feat half is reused by pass 2, since
    agg2 = A[feat] @ Wc_top + A[rs] @ Wc_bot).
  - Tail A: zr = sigmoid(aggT.T @ Wzr + bzr); z^T persisted; rs = r * state
    stored to DRAM in fp8. Every ~12 blocks one AllGather chunk ships rs to
    all cores (4 chunks, each its own DRAM tensor so the chunk collectives
    and the pass-2 gathers that consume them pipeline independently).
  - Pass 2 groups edges by (dst block, src chunk). For each chunk class, as
    soon as its collective lands, single-row rs values are gathered with
    elem_size=256B/elem_step=64B (row + 3 ignored neighbor rows), upconverted
    fp8->bf16, and accumulated via one-hot matmuls into [64, 128] PSUM, then
    added into an SBUF accumulator accT. After a block's last class:
    c^T = tanh(Wc_top^T @ A[f]^T + Wc_bot^T @ accT + bc); new_state^T =
    z^T * (state^T - c^T) + c^T, stored transposed (512B lines, no penalty).
"""

import numpy as np

import concourse.bass as bass
import concourse.bacc as bacc
import concourse.mybir as mybir
import concourse.tile as tile
from concourse.bass_utils import run_bass_kernel_spmd
from concourse.library_config import mlp
from concourse.masks import make_identity

N_NODES = 50000
N_EDGES = 640000
HID = 64
N_CORES = 8
BLK = 128            # dst nodes per block (= PSUM partition dim)
SB_BLOCKS = 4        # dst blocks per super-block (pass-1 gather scope)
MAX_G1 = 26          # cap on groups per dma_gather instruction (pass 1)
MAX_G2 = 26          # cap on groups per dma_gather instruction (pass 2)
CH_SPLIT = [0, 17, 33, 49]   # chunk boundaries in blocks (3 chunks)
N_CH = 3

F32 = mybir.dt.float32
BF16 = mybir.dt.bfloat16
FP8 = mybir.dt.float8e4
I16 = mybir.dt.int16

RS_DT = FP8          # collective payload dtype (upconverted to bf16 pair tables)


def _ceil16(x):
    return max(16, ((int(x) + 15) // 16) * 16)


def _prep_edges(dst, src, edge_weight, n_nodes, n_cores):
    """Partition edges by dst core/block; build pass-1 (src half) and pass-2
    (src chunk) group tables. Returns (tables, plan)."""
    shard = n_nodes // n_cores
    nblk = (shard + BLK - 1) // BLK
    split = n_nodes // 2
    e = len(dst)

    ch_rows = [(CH_SPLIT[c + 1] - CH_SPLIT[c]) * BLK for c in range(N_CH)]
    ch_rows[-1] = shard - CH_SPLIT[N_CH - 1] * BLK  # last chunk partial block
    ch_of_block = np.zeros(nblk, np.int64)
    for c in range(N_CH):
        ch_of_block[CH_SPLIT[c]:CH_SPLIT[c + 1]] = c

    dsts = dst.astype(np.int64)
    srcs = src.astype(np.int64)
    owner = dsts // shard
    local = dsts - owner * shard

    # Balance in-degree across blocks: per core, deal nodes (sorted by
    # in-degree, desc) round-robin over blocks.
    deg = np.zeros(n_nodes, np.int64)
    np.add.at(deg, dsts, 1)
    pos = np.empty((n_cores, shard), np.int64)
    blk_fill = np.empty(nblk, np.int64)
    cap = np.full(nblk, BLK, np.int64)
    cap[nblk - 1] = shard - (nblk - 1) * BLK
    for p in range(n_cores):
        nodes = np.argsort(-deg[p * shard : (p + 1) * shard], kind="stable")
        blk_fill[:] = 0
        bi = 0
        for n in nodes:
            while blk_fill[bi % nblk] >= cap[bi % nblk]:
                bi += 1
            b = bi % nblk
            pos[p, n] = b * BLK + blk_fill[b]
            blk_fill[b] += 1
            bi += 1
    newloc = pos[owner, local]
    b_of = newloc // BLK
    dloc = (newloc % BLK).astype(np.float64)

    src_owner = srcs // shard
    src_l = pos[src_owner, srcs % shard]          # permuted local row of src
    src_pos = src_owner * shard + src_l           # global row in x1b
    cls1 = (src_pos >= split).astype(np.int64)
    src_local1 = src_pos - cls1 * split           # pass-1 idx (< 25000)

    src_ch = ch_of_block[src_l // BLK]            # pass-2 class (src chunk)
    row2 = src_owner * np.array(ch_rows)[src_ch] + (
        src_l - np.array(CH_SPLIT)[src_ch] * BLK
    )                                             # pass-2 idx within chunk tbl

    w64 = edge_weight.astype(np.float64)

    def build_pass(cls, idxval, n_cls, unit_order, force_units, par=None):
        """Group edges into (block, cls) units following unit_order.

        Returns idx16 [cores,16,8*ng], dst_t, w_t [cores,128,ng] (f64),
        chunks [(g0,g1,cls,nidx)], blk_of_g, first_g/last_g dicts keyed
        (b, cls), ngroups.
        """
        cnt = np.zeros((n_cores, nblk, n_cls), np.int64)
        np.add.at(cnt, (owner, b_of, cls), 1)
        unit_max = cnt.max(axis=0)                   # [nblk, n_cls]
        gp = -(-unit_max // BLK)                     # groups per unit
        for (b, c) in force_units:
            gp[b, c] = max(gp[b, c], 1)

        unit_sizes = np.array([gp[b, c] for (b, c) in unit_order], np.int64)
        unit_off = np.concatenate([[0], np.cumsum(unit_sizes)])
        ngroups = int(unit_off[-1])
        unit_idx = {bc: i for i, bc in enumerate(unit_order)}

        blk_of_g = np.zeros(ngroups, np.int64)
        cls_of_u = {}
        for i, (b, c) in enumerate(unit_order):
            blk_of_g[unit_off[i] : unit_off[i + 1]] = b
            cls_of_u[(b, c)] = i
        first_g = {}
        last_g = {}
        for i, (b, c) in enumerate(unit_order):
            if gp[b, c] > 0:
                first_g[(b, c)] = int(unit_off[i])
                last_g[(b, c)] = int(unit_off[i + 1]) - 1

        # number of real idxs in each unit (rounded up to 16)
        unit_n16 = np.array(
            [_ceil16(unit_max[b, c]) if gp[b, c] > 0 else 0 for (b, c) in unit_order],
            np.int64,
        )
        # clamp to group capacity
        unit_n16 = np.minimum(unit_n16, unit_sizes * BLK)

        # gather chunks: runs of same class in unit_order, capped; slots past
        # the last real edge of a chunk's tail unit are trimmed off num_idxs
        # (un-gathered slots stay stale in SBUF; their one-hot weights are 0).
        maxg = MAX_G1 if n_cls == 2 else MAX_G2
        chunks = []
        i = 0
        while i < len(unit_order):
            c = unit_order[i][1]
            j = i
            while j < len(unit_order) and unit_order[j][1] == c:
                j += 1
            g_run1 = int(unit_off[j]) if j < len(unit_order) else ngroups
            s = int(unit_off[i])
            while s < g_run1:
                t = min(s + maxg, g_run1)
                u = int(np.searchsorted(unit_off, t - 1, side="right") - 1)
                if t == unit_off[u + 1]:
                    # chunk ends at unit u's end: drop u's tail padding
                    lu0 = max(int(unit_off[u]), s)
                    done_before = (lu0 - int(unit_off[u])) * BLK
                    tail = int(unit_n16[u]) - done_before
                    tail = max(16, min(tail, (t - lu0) * BLK))
                    nidx = (lu0 - s) * BLK + tail
                else:
                    nidx = (t - s) * BLK
                chunks.append((int(s), int(t), int(c), int(_ceil16(nidx))))
                s = t
            i = j

        # slot assignment: rank within (core, unit)
        ukey = np.array([unit_idx[(b, c)] for b, c in zip(b_of, cls)], np.int64)
        ck = owner * len(unit_order) + ukey
        order2 = np.argsort(ck, kind="stable")
        ck_s = ck[order2]
        owner_s = owner[order2]
        idx_s = idxval[order2]
        ukey_s = ukey[order2]
        dloc_s = dloc[order2]
        ws_s = w64[order2]
        bucket_start = np.searchsorted(ck_s, np.arange(n_cores * len(unit_order)))
        rank = np.arange(e) - bucket_start[ck_s]
        g_global = unit_off[ukey_s] + rank // BLK
        lane = rank % BLK

        idx16 = np.zeros((n_cores, 16, 8 * ngroups), np.int16)
        dst_t = np.zeros((n_cores, BLK, ngroups), np.float64)
        w_t = np.zeros((n_cores, BLK, ngroups), np.float64)
        idx16[owner_s, lane % 16, 8 * g_global + lane // 16] = idx_s.astype(np.int16)
        if par is None:
            dst_t[owner_s, lane, g_global] = dloc_s
        else:
            dst_t[owner_s, lane, g_global] = dloc_s + BLK * par[order2]
        w_t[owner_s, lane, g_global] = ws_s
        return {
            "idx16": idx16,
            "dst_t": dst_t,
            "w_t": w_t,
            "chunks": chunks,
            "blk_of_g": [int(x) for x in blk_of_g],
            "first_g": first_g,
            "last_g": last_g,
            "ngroups": ngroups,
            "gp": gp,
        }

    # ---- pass 1: units (sb, half, block) ----
    unit_order1 = []
    for sb0 in range(0, nblk, SB_BLOCKS):
        sbb = range(sb0, min(sb0 + SB_BLOCKS, nblk))
        for c in range(2):
            for b in sbb:
                unit_order1.append((b, c))
    force1 = [(b, 0) for b in range(nblk)]
    p1 = build_pass(cls1, src_local1, 2, unit_order1, force1)

    # ---- pass 2 (packed): slots packed at 16-granularity; groups span
    # blocks, with one one-hot emission per (group, block) segment ----
    def build_pass2_packed(cls, idxval, par):
        cnt = np.zeros((n_cores, nblk, N_CH), np.int64)
        np.add.at(cnt, (owner, b_of, cls), 1)
        unit_max = cnt.max(axis=0)                  # [nblk, N_CH]
        n16 = np.zeros((nblk, N_CH), np.int64)
        for b in range(nblk):
            for c in range(N_CH):
                if unit_max[b, c] > 0 or c == 0:
                    n16[b, c] = _ceil16(unit_max[b, c])

        unit_order = []
        for c in range(N_CH):
            for sb0 in range(0, nblk, SB_BLOCKS):
                for b in range(sb0, min(sb0 + SB_BLOCKS, nblk)):
                    if n16[b, c] > 0:
                        unit_order.append((b, c))
        # pack units; round each class's span up to 128 slots
        unit_base = {}
        slot = 0
        cls_span = {}
        cur_c = None
        for (b, c) in unit_order + [(None, None)]:
            if c != cur_c:
                if cur_c is not None:
                    slot = -(-slot // BLK) * BLK
                    cls_span[cur_c] = (cls_span[cur_c][0], slot)
                if c is not None:
                    cls_span[c] = (slot, None)
                cur_c = c
            if b is not None:
                unit_base[(b, c)] = slot
                slot += n16[b, c]
        nslots = slot
        ngroups = nslots // BLK

        # emissions: per group, per (block-segment)
        emit_of = {}
        emissions = [[] for _ in range(ngroups)]
        n_emit = 0
        for (b, c) in unit_order:
            s0, s1 = unit_base[(b, c)], unit_base[(b, c)] + n16[b, c]
            for g in range(s0 // BLK, (s1 - 1) // BLK + 1):
                emit_of[(g, b)] = n_emit
                emissions[g].append([b, c, n_emit, False, False])
                n_emit += 1
        # start/stop flags per (b, c)
        for (b, c) in unit_order:
            s0, s1 = unit_base[(b, c)], unit_base[(b, c)] + n16[b, c]
            g0u, g1u = s0 // BLK, (s1 - 1) // BLK
            for em in emissions[g0u]:
                if em[0] == b and em[1] == c:
                    em[3] = True
            for em in emissions[g1u]:
                if em[0] == b and em[1] == c:
                    em[4] = True

        # gather chunks per class, trimmed at class tails
        chunks = []
        for c in range(N_CH):
            s0, s1 = cls_span[c]
            real_end = max(
                unit_base[(b, cc)] + n16[b, cc]
                for (b, cc) in unit_order
                if cc == c
            )
            g0c, g1c = s0 // BLK, s1 // BLK
            s = g0c
            while s < g1c:
                t = min(s + MAX_G2, g1c)
                nidx = min(t * BLK, real_end) - s * BLK
                chunks.append((s, t, c, int(_ceil16(max(16, nidx)))))
                s = t

        # slot assignment: rank within (core, unit)
        uidx = {bc: i for i, bc in enumerate(unit_order)}
        ukey = np.array([uidx[(b, c)] for b, c in zip(b_of, cls)], np.int64)
        ck = owner * len(unit_order) + ukey
        order2 = np.argsort(ck, kind="stable")
        ck_s = ck[order2]
        owner_s = owner[order2]
        idx_s = idxval[order2]
        dloc_s = dloc[order2]
        ws_s = w64[order2]
        par_s = par[order2]
        base_arr = np.array([unit_base[bc] for bc in unit_order], np.int64)
        bucket_start = np.searchsorted(ck_s, np.arange(n_cores * len(unit_order)))
        rank = np.arange(e) - bucket_start[ck_s]
        slot_arr = base_arr[ukey_s := ukey[order2]] + rank
        g_arr = slot_arr // BLK
        lane = slot_arr % BLK
        col = np.array(
            [emit_of[(int(g), int(b))] for g, b in zip(g_arr, b_of[order2])],
            np.int64,
        )

        idx16 = np.zeros((n_cores, 16, 8 * ngroups), np.int16)
        dst_t = np.zeros((n_cores, BLK, n_emit), np.float64)
        w_t = np.zeros((n_cores, BLK, n_emit), np.float64)
        idx16[owner_s, lane % 16, 8 * g_arr + lane // 16] = idx_s.astype(np.int16)
        dst_t[owner_s, lane, col] = dloc_s + BLK * par_s
        w_t[owner_s, lane, col] = ws_s
        return {
            "idx16": idx16,
            "dst_t": dst_t,
            "w_t": w_t,
            "chunks": chunks,
            "emissions": emissions,
            "ngroups": ngroups,
            "n_emit": n_emit,
            "n16": n16,
        }

    p2 = build_pass2_packed(src_ch, row2 // 2, (row2 % 2))

    last_cls = np.zeros(nblk, np.int64)
    for b in range(nblk):
        for c in range(N_CH):
            if p2["n16"][b, c] > 0:
                last_cls[b] = c

    plan = {
        "p1": p1,
        "p2": p2,
        "nblk": nblk,
        "shard": shard,
        "ch_rows": ch_rows,
        "last_cls": [int(x) for x in last_cls],
        "pos": pos,
    }
    return plan


def _build(n_nodes, hid, plan, n_cores, n_queues=4):
    """Build the SPMD Bass program from the edge plan."""
    shard = plan["shard"]
    nblk = plan["nblk"]
    p1, p2 = plan["p1"], plan["p2"]
    ch_rows = plan["ch_rows"]
    last_cls = plan["last_cls"]
    split = n_nodes // 2
    h2 = 2 * hid
    ng1, ng2 = p1["ngroups"], p2["ngroups"]
    n_emit = p2["n_emit"]
    npad = nblk * BLK  # 6272

    nc = bacc.Bacc(None, num_devices=n_cores, num_swdge_queues=n_queues)

    x1b = nc.dram_tensor("x1b", [n_nodes, h2], BF16, kind="ExternalInput")
    st_d = nc.dram_tensor("st_d", [npad, hid], BF16, kind="ExternalInput")
    stT_d = nc.dram_tensor("stT_d", [hid, npad], BF16, kind="ExternalInput")
    idx1_d = nc.dram_tensor("idx1", [BLK, 8 * ng1], I16, kind="ExternalInput")
    idx2_d = nc.dram_tensor("idx2", [BLK, 8 * ng2], I16, kind="ExternalInput")
    dst1_d = nc.dram_tensor("dst1", [BLK, ng1], F32, kind="ExternalInput")
    w1_d = nc.dram_tensor("w1", [BLK, ng1], F32, kind="ExternalInput")
    dst2_d = nc.dram_tensor("dst2", [BLK, n_emit], F32, kind="ExternalInput")
    w2_d = nc.dram_tensor("w2", [BLK, n_emit], F32, kind="ExternalInput")
    wzr = nc.dram_tensor("wzr", [h2, h2], F32, kind="ExternalInput")
    bzr = nc.dram_tensor("bzr", [1, h2], F32, kind="ExternalInput")
    wc = nc.dram_tensor("wc", [h2, hid], F32, kind="ExternalInput")
    bc = nc.dram_tensor("bc", [1, hid], F32, kind="ExternalInput")
    outT = nc.dram_tensor("outT", [hid, npad], F32, kind="ExternalOutput")

    rs_sh = [
        nc.dram_tensor(f"rs_sh{c}", [ch_rows[c], hid], RS_DT, kind="Internal")
        for c in range(N_CH)
    ]
    ch_real = [n_cores * ch_rows[c] for c in range(N_CH)]
    ch_alloc = [-(-r // BLK) * BLK for r in ch_real]
    rs_full = [
        nc.dram_tensor(
            f"rs_full{c}", [ch_alloc[c], hid], RS_DT,
            kind="Internal", addr_space="Shared",
        )
        for c in range(N_CH)
    ]
    rs_pair = [
        nc.dram_tensor(
            f"rs_pair{c}", [ch_alloc[c] + 2, hid], BF16, kind="Internal"
        )
        for c in range(N_CH)
    ]

    qn = [0]

    def next_q():
        q = qn[0]
        qn[0] = (qn[0] + 1) % n_queues
        return q

    def rows_of(b):
        return BLK if b < nblk - 1 else shard - (nblk - 1) * BLK

    with tile.TileContext(nc) as tc:
        with (
            tc.tile_pool(name="const", bufs=1) as const_pool,
            tc.tile_pool(name="store", bufs=1) as store_pool,
            tc.tile_pool(name="msg", bufs=3) as msg_pool,
            tc.tile_pool(name="oh", bufs=6) as oh_pool,
            tc.tile_pool(name="blk", bufs=6) as blk_pool,
            tc.tile_pool(name="stage", bufs=2) as stage_pool,
            tc.tile_pool(name="agg_ps", bufs=4, space="PSUM") as agg_psum,
            tc.tile_pool(name="agg2_ps", bufs=2, space="PSUM") as agg2_psum,
            tc.tile_pool(name="mm_ps", bufs=2, space="PSUM") as mm_psum,
        ):
            nc.gpsimd.load_library(mlp)
            # ---- constants ----
            iota_i = const_pool.tile([BLK, BLK], mybir.dt.int32)
            nc.gpsimd.iota(iota_i[:], pattern=[[1, BLK]], base=0, channel_multiplier=0)
            iota_h = const_pool.tile([BLK, BLK], BF16)
            nc.vector.tensor_copy(iota_h[:], iota_i[:])
            iota2_i = const_pool.tile([BLK, 2 * BLK], mybir.dt.int32)
            nc.gpsimd.iota(
                iota2_i[:], pattern=[[1, 2 * BLK]], base=0, channel_multiplier=0
            )
            iota2_h = const_pool.tile([BLK, 2 * BLK], BF16)
            nc.vector.tensor_copy(iota2_h[:], iota2_i[:])
            identity = const_pool.tile([BLK, BLK], F32)
            make_identity(nc, identity[:])
            ones1 = const_pool.tile([1, BLK], F32)
            nc.vector.memset(ones1[:], 1.0)
            wzr_sb = const_pool.tile([h2, h2], F32)
            nc.sync.dma_start(out=wzr_sb[:], in_=wzr[:, :])
            bzr_sb = const_pool.tile([1, h2], F32)
            nc.sync.dma_start(out=bzr_sb[:], in_=bzr[:, :])
            wctop_sb = const_pool.tile([hid, hid], F32)
            nc.sync.dma_start(out=wctop_sb[:], in_=wc[0:hid, :])
            wcbot_sb = const_pool.tile([hid, hid], F32)
            nc.sync.dma_start(out=wcbot_sb[:], in_=wc[hid:h2, :])
            bc_sb = const_pool.tile([1, hid], F32)
            nc.sync.dma_start(out=bc_sb[:], in_=bc[:, :])

            # ---- persistent tables (pass-1 quarters first so gathers can
            # start ~2us in; the rest stream behind them on the DMA queue) ----
            idx1_sb = store_pool.tile([BLK, 8 * ng1], I16)
            dst1_sb = store_pool.tile([BLK, ng1], F32)
            w1_sb = store_pool.tile([BLK, ng1], F32)

            def rep128(tile_sb, c0, c1):
                pass

            q1 = [0] + [min(ng1, (ng1 * (q + 1)) // 4) for q in range(4)]
            nc.sync.dma_start(
                out=idx1_sb[:, : 8 * q1[1]], in_=idx1_d[:, : 8 * q1[1]]
            )
            nc.sync.dma_start(out=dst1_sb[:, : q1[1]], in_=dst1_d[:, : q1[1]])
            nc.sync.dma_start(out=w1_sb[:, : q1[1]], in_=w1_d[:, : q1[1]])

            st_store = store_pool.tile([BLK, nblk * hid], BF16)
            nc.vector.memset(st_store[:], 0.0)
            stT_store = store_pool.tile([hid, npad], BF16)

            def late_loads2(ci):
                if ci == 2:
                    nc.sync.dma_start(
                        out=st_store[:].rearrange("l (b h) -> l b h", h=hid),
                        in_=st_d[:, :].rearrange("(b l) h -> l b h", l=BLK),
                    )
                elif ci == 12:
                    nc.sync.dma_start(out=stT_store[:], in_=stT_d[:, :])
            idx2_sb = store_pool.tile([BLK, 8 * ng2], I16)
            dst2_sb = store_pool.tile([BLK, n_emit], F32)
            w2_sb = store_pool.tile([BLK, n_emit], F32)

            def late_loads(ci):
                if ci in (1, 3, 5):
                    q = (ci + 1) // 2
                    a, b = q1[q], q1[q + 1]
                    nc.sync.dma_start(
                        out=idx1_sb[:, 8 * a : 8 * b], in_=idx1_d[:, 8 * a : 8 * b]
                    )
                    nc.sync.dma_start(out=dst1_sb[:, a:b], in_=dst1_d[:, a:b])
                    nc.sync.dma_start(out=w1_sb[:, a:b], in_=w1_d[:, a:b])
                elif ci == 14:
                    nc.sync.dma_start(out=idx2_sb[:], in_=idx2_d[:, :])
                elif ci == 16:
                    nc.sync.dma_start(out=dst2_sb[:], in_=dst2_d[:, :])
                    nc.sync.dma_start(out=w2_sb[:], in_=w2_d[:, :])

            zpad = const_pool.tile([BLK, hid], RS_DT)
            nc.vector.memset(zpad[:], 0.0)
            for c in range(N_CH):
                if ch_alloc[c] > ch_real[c]:
                    nc.sync.dma_start(
                        out=rs_full[c][ch_real[c] : ch_alloc[c], :],
                        in_=zpad[: ch_alloc[c] - ch_real[c], :],
                    )

            aggT_store = store_pool.tile([h2, npad], F32)
            accT = store_pool.tile([hid, npad], F32)
            zT_store = store_pool.tile([hid, npad], F32)

            psum_of = {}

            def do_collective(ch):
                nc.gpsimd.collective_compute(
                    "AllGather",
                    mybir.AluOpType.bypass,
                    replica_groups=[list(range(n_cores))],
                    ins=[rs_sh[ch][:, :]],
                    outs=[rs_full[ch][0 : ch_real[ch], :]],
                )

            # ============== Phase A: pass-1 aggregation ===============
            def drain_a(b):
                nc.scalar.activation(
                    aggT_store[:, b * BLK : (b + 1) * BLK],
                    psum_of.pop(b)[:],
                    mybir.ActivationFunctionType.Copy,
                )

            def tail_a(b):
                R = rows_of(b)
                zr_ps = mm_psum.tile([BLK, h2], F32, tag="mm")
                nc.tensor.matmul(
                    zr_ps[:],
                    lhsT=aggT_store[:, b * BLK : (b + 1) * BLK],
                    rhs=wzr_sb[:],
                    start=True,
                    stop=False,
                )
                nc.tensor.matmul(
                    zr_ps[:], lhsT=ones1[:], rhs=bzr_sb[:], start=False, stop=True
                )
                zr_sb = blk_pool.tile([BLK, h2], F32, tag="zr")
                nc.scalar.activation(
                    zr_sb[:], zr_ps[:], mybir.ActivationFunctionType.Sigmoid
                )
                ztp = mm_psum.tile([hid, BLK], F32, tag="mm")
                nc.tensor.transpose(
                    out=ztp[:], in_=zr_sb[:, 0:hid], identity=identity[:]
                )
                nc.scalar.activation(
                    zT_store[:, b * BLK : (b + 1) * BLK],
                    ztp[:],
                    mybir.ActivationFunctionType.Copy,
                )
                rs = blk_pool.tile([BLK, hid], RS_DT, tag="rs")
                nc.vector.tensor_tensor(
                    out=rs[:],
                    in0=zr_sb[:, hid:h2],
                    in1=st_store[:, b * hid : (b + 1) * hid],
                    op=mybir.AluOpType.mult,
                )
                ch = 0
                while b >= CH_SPLIT[ch + 1]:
                    ch += 1
                r0 = (b - CH_SPLIT[ch]) * BLK
                nc.scalar.dma_start(out=rs_sh[ch][r0 : r0 + R, :], in_=rs[:R, :])
                if b == CH_SPLIT[ch + 1] - 1:
                    do_collective(ch)

            blk1 = p1["blk_of_g"]
            f1 = {}
            l1 = {}
            for b in range(nblk):
                gs = [
                    p1["first_g"].get((b, c)) for c in range(2) if (b, c) in p1["first_g"]
                ]
                ge = [
                    p1["last_g"].get((b, c)) for c in range(2) if (b, c) in p1["last_g"]
                ]
                f1[b] = min(gs)
                l1[b] = max(ge)

            pend_a = []
            for ci, (g0, g1, c, nidx) in enumerate(p1["chunks"]):
                kg = g1 - g0
                if ci < 3:
                    nidx = kg * BLK
                late_loads(ci)
                late_loads2(ci)
                tbl = x1b[0:split, :] if c == 0 else x1b[split:n_nodes, :]
                msgs = msg_pool.tile([BLK, MAX_G1 * h2], BF16, tag="m1")
                out_ap = msgs[:, : kg * h2].rearrange("p (t w) -> p t w", w=h2)
                nc.gpsimd.dma_gather(
                    out_ap,
                    tbl,
                    idx1_sb[:, 8 * g0 : 8 * g0 + nidx // 16],
                    nidx,
                    nidx,
                    h2,
                    queue_num=next_q(),
                    single_packet=False,
                )
                for g in range(g0, g1):
                    b = blk1[g]
                    if b not in psum_of:
                        psum_of[b] = agg_psum.tile(
                            [h2, BLK], F32, tag="agg", name=f"agga{b}"
                        )
                    oh = oh_pool.tile([BLK, BLK], BF16, tag="oh")
                    nc.vector.tensor_scalar(
                        out=oh[:],
                        in0=iota_h[:],
                        scalar1=dst1_sb[:, g : g + 1],
                        scalar2=w1_sb[:, g : g + 1],
                        op0=mybir.AluOpType.is_equal,
                        op1=mybir.AluOpType.mult,
                    )
                    gl = (g - g0) * h2
                    nc.tensor.matmul(
                        out=psum_of[b][:],
                        lhsT=msgs[:, gl : gl + h2],
                        rhs=oh[:],
                        start=(g == f1[b]),
                        stop=(g == l1[b]),
                    )
                    if g == l1[b]:
                        drain_a(b)
                        pend_a.append(b)
                        if b in (CH_SPLIT[1] - 1, CH_SPLIT[2] - 1, nblk - 1):
                            while pend_a:
                                tail_a(pend_a.pop(0))
                        elif len(pend_a) > 2 * SB_BLOCKS:
                            tail_a(pend_a.pop(0))
            while pend_a:
                tail_a(pend_a.pop(0))

            # ---- upconvert fp8 rs_full -> bf16 pair tables (Act converts;
            # SP stalls on collective sems are harmless here) ----
            for c in range(N_CH):
                qs = [0]
                for q in range(8):
                    qs.append((ch_alloc[c] * (q + 1) // 8 // BLK) * BLK)
                qs[-1] = ch_alloc[c]
                for q in range(8):
                    r0, r1 = qs[q], qs[q + 1]
                    w = (r1 - r0) * hid // BLK
                    sf8 = stage_pool.tile([BLK, 2176], RS_DT, tag="sf8")
                    nc.sync.dma_start(
                        out=sf8[:, :w],
                        in_=rs_full[c][r0:r1, :].rearrange(
                            "(p r) h -> p (r h)", p=BLK
                        ),
                    )
                    sbf = stage_pool.tile([BLK, 2176], BF16, tag="sbf")
                    nc.scalar.activation(
                        sbf[:, :w], sf8[:, :w], mybir.ActivationFunctionType.Copy
                    )
                    nc.sync.dma_start(
                        out=rs_pair[c][r0:r1, :].rearrange(
                            "(p r) h -> p (r h)", p=BLK
                        ),
                        in_=sbf[:, :w],
                    )

            # ============== Phase C: pass-2 aggregation + output ===========
            def tail_c(b):
                R = rows_of(b)
                cps = mm_psum.tile([hid, BLK], F32, tag="mm")
                nc.tensor.matmul(
                    cps[:],
                    lhsT=wctop_sb[:],
                    rhs=aggT_store[0:hid, b * BLK : (b + 1) * BLK],
                    start=True,
                    stop=False,
                )
                nc.tensor.matmul(
                    cps[:],
                    lhsT=wcbot_sb[:],
                    rhs=accT[:, b * BLK : (b + 1) * BLK],
                    start=False,
                    stop=False,
                )
                nc.tensor.matmul(
                    cps[:], lhsT=bc_sb[:], rhs=ones1[:], start=False, stop=True
                )
                cT = blk_pool.tile([hid, BLK], F32, tag="cT")
                nc.scalar.activation(
                    cT[:], cps[:], mybir.ActivationFunctionType.Tanh
                )
                t1 = blk_pool.tile([hid, BLK], F32, tag="t1")
                nc.vector.tensor_tensor(
                    out=t1[:],
                    in0=stT_store[:, b * BLK : (b + 1) * BLK],
                    in1=cT[:],
                    op=mybir.AluOpType.subtract,
                )
                t2 = blk_pool.tile([hid, BLK], F32, tag="t2")
                nc.gpsimd.tensor_tensor(
                    out=t2[:],
                    in0=t1[:],
                    in1=zT_store[:, b * BLK : (b + 1) * BLK],
                    op=mybir.AluOpType.mult,
                )
                nsT = blk_pool.tile([hid, BLK], F32, tag="nsT")
                nc.vector.tensor_tensor(
                    out=nsT[:], in0=t2[:], in1=cT[:], op=mybir.AluOpType.add
                )
                nc.scalar.dma_start(
                    out=outT[:, b * BLK : b * BLK + R], in_=nsT[:, :R]
                )

            emissions = p2["emissions"]
            psum2 = {}
            pend_c = []
            acc_init = set()

            for ci, (g0, g1, c2, nidx) in enumerate(p2["chunks"]):
                kg = g1 - g0
                if ci < 3:
                    nidx = kg * BLK
                msgs2 = msg_pool.tile([BLK, MAX_G2 * h2], BF16, tag="m2")
                out_ap = msgs2[:, : kg * h2].rearrange("p (t w) -> p t w", w=h2)
                nc.gpsimd.dma_gather(
                    out_ap,
                    rs_pair[c2][:, :].rearrange("(a b) h -> a (b h)", b=2),
                    idx2_sb[:, 8 * g0 : 8 * g0 + nidx // 16],
                    nidx,
                    nidx,
                    h2,
                    queue_num=next_q(),
                    single_packet=False,
                )
                for g in range(g0, g1):
                    gl = (g - g0) * h2
                    for b, ec, col, e_start, e_stop in emissions[g]:
                        key = (b, ec)
                        if key not in psum2:
                            psum2[key] = agg2_psum.tile(
                                [hid, BLK], F32, tag="agg2", name=f"aggc{b}_{ec}"
                            )
                        oh = oh_pool.tile([BLK, 2 * BLK], BF16, tag="oh2")
                        nc.vector.tensor_scalar(
                            out=oh[:],
                            in0=iota2_h[:],
                            scalar1=dst2_sb[:, col : col + 1],
                            scalar2=w2_sb[:, col : col + 1],
                            op0=mybir.AluOpType.is_equal,
                            op1=mybir.AluOpType.mult,
                        )
                        nc.tensor.matmul(
                            out=psum2[key][:],
                            lhsT=msgs2[:, gl : gl + hid],
                            rhs=oh[:, 0:BLK],
                            start=e_start,
                            stop=False,
                        )
                        nc.tensor.matmul(
                            out=psum2[key][:],
                            lhsT=msgs2[:, gl + hid : gl + h2],
                            rhs=oh[:, BLK : 2 * BLK],
                            start=False,
                            stop=e_stop,
                        )
                        if e_stop:
                            ps = psum2.pop(key)
                            if b not in acc_init:
                                acc_init.add(b)
                                nc.scalar.activation(
                                    accT[:, b * BLK : (b + 1) * BLK],
                                    ps[:],
                                    mybir.ActivationFunctionType.Copy,
                                )
                            else:
                                nc.vector.tensor_tensor(
                                    out=accT[:, b * BLK : (b + 1) * BLK],
                                    in0=ps[:],
                                    in1=accT[:, b * BLK : (b + 1) * BLK],
                                    op=mybir.AluOpType.add,
                                )
                            if ec == last_cls[b]:
                                pend_c.append(b)
                                if len(pend_c) > 8:
                                    tail_c(pend_c.pop(0))
            while pend_c:
                tail_c(pend_c.pop(0))

    nc.finalize()
    return nc


def run(feat, state, src, dst, edge_weight, Wzr, bzr, Wc, bc, trace=False):
    """Build + run on 8 cores; returns (new_state, BassKernelResults)."""
    import ml_dtypes

    n_nodes, hid = feat.shape
    n_cores = N_CORES
    shard = n_nodes // n_cores

    plan = _prep_edges(dst, src, edge_weight, n_nodes, n_cores)
    pos = plan["pos"]
    nblk = plan["nblk"]
    npad = nblk * BLK
    p1, p2 = plan["p1"], plan["p2"]

    # global permutation: node (p, l) lives at row p*shard + pos[p, l]
    inv = np.empty((n_cores, shard), np.int64)
    for p in range(n_cores):
        inv[p, pos[p]] = np.arange(shard)
    x1 = np.concatenate([feat, state], axis=1)
    x1p = np.empty_like(x1)
    for p in range(n_cores):
        x1p[p * shard : (p + 1) * shard] = x1[p * shard : (p + 1) * shard][inv[p]]
    x1b = np.ascontiguousarray(x1p.astype(ml_dtypes.bfloat16))

    nc = _build(n_nodes, hid, plan, n_cores)

    in_maps = []
    for p in range(n_cores):
        st_p = state[p * shard : (p + 1) * shard][inv[p]].astype(ml_dtypes.bfloat16)
        st_pad = np.zeros((npad, hid), ml_dtypes.bfloat16)
        st_pad[:shard] = st_p
        stT_pad = np.zeros((hid, npad), ml_dtypes.bfloat16)
        stT_pad[:, :shard] = st_p.T
        in_maps.append(
            {
                "x1b": x1b,
                "st_d": np.ascontiguousarray(st_pad),
                "stT_d": np.ascontiguousarray(stT_pad),
                "idx1": np.ascontiguousarray(np.tile(p1["idx16"][p], (8, 1))),
                "idx2": np.ascontiguousarray(np.tile(p2["idx16"][p], (8, 1))),
                "dst1": np.ascontiguousarray(p1["dst_t"][p].astype(np.float32)),
                "w1": np.ascontiguousarray(p1["w_t"][p].astype(np.float32)),
                "dst2": np.ascontiguousarray(p2["dst_t"][p].astype(np.float32)),
                "w2": np.ascontiguousarray(p2["w_t"][p].astype(np.float32)),
                "wzr": np.ascontiguousarray(Wzr, dtype=np.float32),
                "bzr": np.ascontiguousarray(bzr.reshape(1, -1), dtype=np.float32),
                "wc": np.ascontiguousarray(Wc, dtype=np.float32),
                "bc": np.ascontiguousarray(bc.reshape(1, -1), dtype=np.float32),
            }
        )

    res = run_bass_kernel_spmd(
        nc, in_maps, core_ids=list(range(n_cores)), trace=trace
    )
    shards = [
        res.results[p]["outT"][:, :shard].T[pos[p]] for p in range(n_cores)
    ]
    return np.concatenate(shards, axis=0), res


def kernel(feat, state, src, dst, edge_weight, Wzr, bzr, Wc, bc):
    out, _ = run(feat, state, src, dst, edge_weight, Wzr, bzr, Wc, bc, trace=False)
    return out
